# revision 26
# baseline (speedup 1.0000x reference)
"""Trainium2 Bass kernel for nn_Discriminator2 (bilinear discriminator scores).

Math: with hc0 = h_c[0] [N, D], W0 = W[0] [D, D]:
    v      = hc0 @ W0.T                      [N, D]   (tensor engine, bf16)
    sc1[n] = dot(h_pl[0][n], v[n]) + b       [N]      (fused DVE mult+reduce)
    sc2[s,n] = dot(hc0[sample[s,n]], v[n]) + b        (bulk SWDGE gather + DVE)
    out    = [sc1 | sc2.flat | sc2.flat]     [1, N + 2*S*N]

Sharding: nodes (N) split evenly across 8 cores; W replicated; h_pl /
sample_list sharded by node.

Gather strategy: dma_gather (InstDMAGatherAnt) batches thousands of indices
per Pool-engine call (~1us fixed overhead amortized), but takes int16
indices.  The full 100k-row table is not int16-addressable, so the host
splits each core's nodes into two halves and builds a per-half COMPACT
table holding only the ~22k unique rows that half references (remapped
int16 indices).  Each unique row is uploaded once; every per-sample random
fetch still happens on-device from the compact table.

Per-core device pipeline (12500 nodes = 98 tiles of 128, 2 halves x 49):
  - one dma_gather per 2-tile block (1024 indices — the SWDGE ring holds
    1024 descriptors, larger calls hard-fault; 4 queues round-robin so
    descriptor generation never stalls on ring drain; slot i ->
    (i%128, i//128) matches the (tile, stream) layout exactly)
  - hcT/hpl stream in bf16 via HWDGE; v = hc0 @ W.T in bf16 on the PE
  - scalar engine converts v (PSUM fp32) to bf16 once per tile
  - dot products split across engines (no fused multiply-reduce exists on
    this runtime): DVE multiplies all 5 streams (broadcast 2x pass for the
    4 gathered ones) and segment-reduces 3; ACT reduces the other 2 via
    Copy+accumulate.  Pool runs ONLY gathers — interleaving any tensor op
    there costs ~1us MODIFY_POOL_CONFIG per switch.
"""

import sys

for _p in ("/opt/trn_rl_repo",):
    if _p not in sys.path:
        sys.path.insert(0, _p)

import ml_dtypes
import numpy as np

import concourse.mybir as mybir
import concourse.tile as tile
from concourse import bacc
from concourse.bass_utils import run_bass_kernel_spmd

P = 128  # partitions

_NP_DT = {
    mybir.dt.bfloat16: ml_dtypes.bfloat16,
    mybir.dt.float8e3: ml_dtypes.float8_e3m4,
    mybir.dt.float8e4: ml_dtypes.float8_e4m3,
    mybir.dt.float32: np.float32,
}


class Cfg:
    """Problem geometry. Full-size defaults; shrink for CoreSim validation."""

    def __init__(self, n_table=100000, nodes_per_core=12500, d=512, s=4,
                 n_cores=8, call_tiles=2, table_pad=32768,
                 mm_dtype=mybir.dt.bfloat16,      # matmul operands (hcT, W)
                 g_dtype=mybir.dt.bfloat16,       # gather table
                 hpl_dtype=mybir.dt.bfloat16):    # h_pl stream
        self.n_table = n_table
        self.nodes_per_core = nodes_per_core
        self.d = d
        self.s = s
        self.n_cores = n_cores
        self.call_tiles = call_tiles    # node-tiles per dma_gather call
        self.table_pad = table_pad      # compact-table row allocation
        self.mm_dtype = mm_dtype
        self.g_dtype = g_dtype
        self.hpl_dtype = hpl_dtype
        self.tiles = -(-nodes_per_core // P)        # ceil
        self.npad = self.tiles * P
        self.kc = d // P                # contraction chunks
        # two halves of tiles; each must dedup below int16 range
        self.half_tiles = -(-self.tiles // 2)
        self.idx_cols_half = self.half_tiles * s * P // 16

    def blocks(self):
        """(t0_local, n_tiles) call blocks within one half."""
        out = []
        t = 0
        while t < self.half_tiles:
            nt = min(self.call_tiles, self.half_tiles - t)
            out.append((t, nt))
            t += nt
        return out


FULL = Cfg()


def build_nc(cfg: Cfg):
    D, S, KC, ST = cfg.d, cfg.s, cfg.kc, cfg.call_tiles
    mmdt, gdt, pldt = cfg.mm_dtype, cfg.g_dtype, cfg.hpl_dtype
    f32 = mybir.dt.float32

    nc = bacc.Bacc("TRN2", target_bir_lowering=False, debug=False,
                   num_swdge_queues=4)
    hcA = nc.dram_tensor("hcA", [cfg.table_pad, D], gdt,
                         kind="ExternalInput").ap()
    hcB = nc.dram_tensor("hcB", [cfg.table_pad, D], gdt,
                         kind="ExternalInput").ap()
    hcT = nc.dram_tensor("hcT", [D, cfg.npad], mmdt, kind="ExternalInput").ap()
    hpl = nc.dram_tensor("hpl", [cfg.npad, D], pldt, kind="ExternalInput").ap()
    idx = nc.dram_tensor("idx", [P, 2 * cfg.idx_cols_half], mybir.dt.int16,
                         kind="ExternalInput").ap()
    wt = nc.dram_tensor("wt", [D, D], mmdt, kind="ExternalInput").ap()
    bb = nc.dram_tensor("bb", [P, 1], f32, kind="ExternalInput").ap()
    out = nc.dram_tensor("out", [P, cfg.tiles * (S + 1)], f32,
                         kind="ExternalOutput").ap()

    with tile.TileContext(nc) as tc:
        with (
            tc.tile_pool(name="const", bufs=1) as cpool,
            tc.tile_pool(name="hcT", bufs=2) as hcT_pool,
            tc.tile_pool(name="hpl", bufs=2) as hpl_pool,
            tc.tile_pool(name="g", bufs=4) as g_pool,
            tc.tile_pool(name="vbf", bufs=8) as v_pool,
            tc.tile_pool(name="prod", bufs=6) as prod_pool,
            tc.tile_pool(name="psum", bufs=8, space="PSUM") as psum_pool,
        ):
            idx_sb = cpool.tile([P, 2 * cfg.idx_cols_half], mybir.dt.int16)
            nc.sync.dma_start(out=idx_sb[:], in_=idx[:])
            # W.T resident: free layout (c, d) — chunk c covers contraction
            # rows c*128..c*128+127.
            wt_sb = cpool.tile([P, KC * D], mmdt)
            nc.sync.dma_start(
                out=wt_sb[:].rearrange("p (c d) -> p c d", c=KC),
                in_=wt.rearrange("(c p) d -> p c d", p=P))
            # per-engine score tiles (ACT writes sc_a, DVE writes sc_d) so
            # the two accumulation streams never alias one tile
            sc_a = cpool.tile([P, cfg.tiles * 2], f32)
            sc_d = cpool.tile([P, cfg.tiles * 3], f32)
            dump = cpool.tile([P, D], mybir.dt.bfloat16)  # discarded ACT out

            call_no = 0
            for half, table in ((0, hcA), (1, hcB)):
                for t0_loc, nt in cfg.blocks():
                    t0 = half * cfg.half_tiles + t0_loc
                    nt = min(nt, cfg.tiles - t0)
                    if nt <= 0:
                        break
                    n_idx = nt * S * P
                    col0 = half * cfg.idx_cols_half + t0_loc * S * P // 16
                    g_sb = g_pool.tile([P, ST * S * D], gdt, tag="g")
                    nc.gpsimd.dma_gather(
                        out_ap=g_sb[:, : nt * S * D].rearrange(
                            "p (k d) -> p k d", d=D),
                        in_ap=table[:],
                        idxs_ap=idx_sb[:, col0:col0 + n_idx // 16],
                        num_idxs=n_idx,
                        num_idxs_reg=n_idx,
                        elem_size=D,
                        queue_num=call_no % 4,
                    )
                    call_no += 1
                    # hcT block [D, nt*128] -> SBUF free layout (c, n_local)
                    hcT_sb = hcT_pool.tile([P, KC * ST * P], mmdt, tag="hcT")
                    nc.sync.dma_start(
                        out=hcT_sb[:, : KC * nt * P].rearrange(
                            "p (c n) -> p c n", c=KC),
                        in_=hcT[:, t0 * P:(t0 + nt) * P].rearrange(
                            "(c p) n -> p c n", p=P),
                    )
                    # h_pl rows for the whole block in one DMA
                    hpl_sb = hpl_pool.tile([P, ST * D], pldt, tag="hpl")
                    nc.sync.dma_start(
                        out=hpl_sb[:, : nt * D].rearrange(
                            "p (t d) -> p t d", t=nt),
                        in_=hpl[t0 * P:(t0 + nt) * P, :].rearrange(
                            "(t p) d -> p t d", p=P),
                    )
                    for j in range(nt):
                        t = t0 + j
                        # v = hc0_tile @ W.T via KC accumulating matmuls
                        v_ps = psum_pool.tile([P, D], f32, space="PSUM",
                                              tag="v_ps")
                        for c in range(KC):
                            off = (c * nt + j) * P
                            nc.tensor.matmul(
                                out=v_ps[:],
                                lhsT=hcT_sb[:, off:off + P],
                                rhs=wt_sb[:, c * D:(c + 1) * D],
                                start=(c == 0),
                                stop=(c == KC - 1),
                            )
                        # bf16 copy of v so the DVE runs in 16-bit mode
                        v_bf = v_pool.tile([P, D], mybir.dt.bfloat16,
                                           tag="v_bf")
                        nc.scalar.activation(
                            v_bf[:], v_ps[:],
                            mybir.ActivationFunctionType.Copy)
                        # DVE: h_pl product (stream 0)
                        prod0 = prod_pool.tile([P, D], mybir.dt.bfloat16,
                                               tag="prod0")
                        nc.vector.tensor_mul(prod0[:],
                                             hpl_sb[:, j * D:(j + 1) * D],
                                             v_bf[:])
                        # DVE: all 4 gathered products in one 2x pass
                        prod4 = prod_pool.tile([P, S * D], mybir.dt.bfloat16,
                                               tag="prod4")
                        nc.vector.tensor_tensor(
                            out=prod4[:].rearrange("p (s d) -> p s d", s=S),
                            in0=g_sb[:, j * S * D:(j + 1) * S * D].rearrange(
                                "p (s d) -> p s d", s=S),
                            in1=v_bf[:].unsqueeze(1).broadcast_to((P, S, D)),
                            op=mybir.AluOpType.mult)
                        # ACT: reduce streams 0 (h_pl) and 1
                        nc.scalar.activation(
                            dump[:], prod0[:],
                            mybir.ActivationFunctionType.Copy,
                            accum_out=sc_a[:, 2 * t:2 * t + 1])
                        nc.scalar.activation(
                            dump[:], prod4[:, :D],
                            mybir.ActivationFunctionType.Copy,
                            accum_out=sc_a[:, 2 * t + 1:2 * t + 2])
                        # DVE: fold streams 2..4 in half at 2x, then the 1x
                        # segmented reduce only sees half the elements
                        fold = prod_pool.tile([P, 3 * (D // 2)],
                                              mybir.dt.bfloat16, tag="fold")
                        h = D // 2
                        nc.vector.tensor_tensor(
                            out=fold[:].rearrange("p (s d) -> p s d", s=3),
                            in0=prod4[:, D:].rearrange(
                                "p (s d) -> p s d", d=D)[:, :, :h],
                            in1=prod4[:, D:].rearrange(
                                "p (s d) -> p s d", d=D)[:, :, h:],
                            op=mybir.AluOpType.add)
                        nc.vector.tensor_reduce(
                            out=sc_d[:, 3 * t:3 * t + 3],
                            in_=fold[:].rearrange("p (s d) -> p s d", s=3),
                            axis=mybir.AxisListType.X,
                            op=mybir.AluOpType.add)
            # bias is added host-side during assembly (exact for any b);
            # keeping it off-device trims the kernel tail
            nc.sync.dma_start(out=out[:, : cfg.tiles * 2], in_=sc_a[:])
            nc.sync.dma_start(out=out[:, cfg.tiles * 2:], in_=sc_d[:])
    nc.compile()
    return nc


def make_in_maps(cfg: Cfg, h_c, h_pl, sample_list, W, b):
    """Host-side sharding: full inputs -> per-core input dicts."""
    D, S = cfg.d, cfg.s
    gnp = _NP_DT[cfg.g_dtype]
    mnp = _NP_DT[cfg.mm_dtype]
    pnp = _NP_DT[cfg.hpl_dtype]
    hc0 = np.asarray(h_c, np.float32)[0]
    hpl0 = np.asarray(h_pl, np.float32)[0]
    smp = np.asarray(sample_list)
    W0 = np.asarray(W, np.float32)[0]
    bval = float(np.asarray(b, np.float32).reshape(-1)[0])

    hc_g = hc0.astype(gnp)                             # quantized gather rows
    hcT = np.ascontiguousarray(hc0.T.astype(mnp))      # [D, N]
    wt = np.ascontiguousarray(W0.T.astype(mnp))        # wt[d, e] = W[e, d]
    b_bcast = np.full((P, 1), bval, np.float32)
    half_nodes = cfg.half_tiles * P

    in_maps = []
    for c in range(cfg.n_cores):
        lo = c * cfg.nodes_per_core
        hi = lo + cfg.nodes_per_core
        hcT_s = np.zeros((D, cfg.npad), mnp)
        hcT_s[:, : cfg.nodes_per_core] = hcT[:, lo:hi]
        hpl_s = np.zeros((cfg.npad, D), pnp)
        hpl_s[: cfg.nodes_per_core] = hpl0[lo:hi].astype(pnp)
        idx_s = np.zeros((S, cfg.npad), np.int64)
        idx_s[:, : cfg.nodes_per_core] = smp[:, lo:hi]

        # per-half compact tables + int16 index lists in call-slot order:
        # flat position ((j*S + s)*128 + p) = sample (s, node j*128+p)
        tables = []
        idx16 = np.zeros((P, 2 * cfg.idx_cols_half), np.int16)
        for h in range(2):
            n0 = h * half_nodes
            n1 = min(n0 + half_nodes, cfg.npad)
            # [tiles_in_half, S, P] -> flat in (j, s, p) order
            m = (idx_s[:, n0:n1].reshape(S, -1, P)
                 .transpose(1, 0, 2).reshape(-1))
            uniq, inv = np.unique(m, return_inverse=True)
            assert len(uniq) <= cfg.table_pad, (c, h, len(uniq))
            tbl = np.zeros((cfg.table_pad, D), gnp)
            tbl[: len(uniq)] = hc_g[uniq]
            tables.append(tbl)
            # pad the half's flat list to the full half_tiles extent
            flat = np.zeros(cfg.half_tiles * S * P, np.int16)
            flat[: len(inv)] = inv.astype(np.int16)
            wrapped = flat.reshape(-1, 16).T                 # [16, cols]
            cs = h * cfg.idx_cols_half
            idx16[:, cs:cs + cfg.idx_cols_half] = np.tile(
                wrapped, (8, 1))
        in_maps.append({
            "hcA": tables[0], "hcB": tables[1], "hcT": hcT_s, "hpl": hpl_s,
            "idx": idx16, "wt": wt, "bb": b_bcast,
        })
    return in_maps


def assemble_output(cfg: Cfg, outs, bval=0.0):
    """Per-core 'out' arrays [P, TILES*(S+1)] -> full logits [1, N + 2*S*N].

    Device layout: out[:, :2T] = sc_a (streams 0,1 per tile),
    out[:, 2T:] = sc_d (streams 2,3,4 per tile)."""
    S = cfg.s
    T = cfg.tiles
    n = cfg.nodes_per_core * cfg.n_cores
    sc1 = np.empty((n,), np.float32)
    sc2 = np.empty((S, n), np.float32)
    for c in range(cfg.n_cores):
        oa = outs[c][:, : 2 * T].reshape(P, T, 2)
        od = outs[c][:, 2 * T:].reshape(P, T, 3)
        o = (np.concatenate([oa, od], axis=2).transpose(2, 1, 0)
             .reshape(S + 1, cfg.npad)[:, : cfg.nodes_per_core])
        lo = c * cfg.nodes_per_core
        sc1[lo:lo + cfg.nodes_per_core] = o[0]
        sc2[:, lo:lo + cfg.nodes_per_core] = o[1:]
    flat = sc2.reshape(-1)
    res = np.concatenate([sc1, flat, flat])[None, :] + np.float32(bval)
    return res.astype(np.float32)


_NC_CACHE = {}


def _get_nc(cfg: Cfg):
    key = (cfg.n_table, cfg.nodes_per_core, cfg.d, cfg.s, cfg.call_tiles,
           cfg.mm_dtype, cfg.g_dtype, cfg.hpl_dtype)
    if key not in _NC_CACHE:
        _NC_CACHE[key] = build_nc(cfg)
    return _NC_CACHE[key]


def run_on_hw(cfg: Cfg, inputs, trace=False, trace_kwargs={}):
    nc = _get_nc(cfg)
    in_maps = make_in_maps(cfg, **inputs)
    res = run_bass_kernel_spmd(nc, in_maps, core_ids=list(range(cfg.n_cores)),
                               trace=trace, trace_kwargs=trace_kwargs)
    bval = float(np.asarray(inputs["b"], np.float32).reshape(-1)[0])
    out = assemble_output(cfg, [r["out"] for r in res.results], bval)
    return out, res


def kernel(h_c, h_pl, sample_list, W, b):
    inputs = dict(h_c=h_c, h_pl=h_pl, sample_list=sample_list, W=W, b=b)
    out, _ = run_on_hw(FULL, inputs, trace=False)
    return out


# revision 27
# speedup vs baseline: 1.0217x; 1.0217x over previous
"""Trainium2 Bass kernel for nn_Discriminator2 (bilinear discriminator scores).

Math: with hc0 = h_c[0] [N, D], W0 = W[0] [D, D]:
    v      = hc0 @ W0.T                      [N, D]   (tensor engine, bf16)
    sc1[n] = dot(h_pl[0][n], v[n]) + b       [N]      (fused DVE mult+reduce)
    sc2[s,n] = dot(hc0[sample[s,n]], v[n]) + b        (bulk SWDGE gather + DVE)
    out    = [sc1 | sc2.flat | sc2.flat]     [1, N + 2*S*N]

Sharding: nodes (N) split evenly across 8 cores; W replicated; h_pl /
sample_list sharded by node.

Gather strategy: dma_gather (InstDMAGatherAnt) batches thousands of indices
per Pool-engine call (~1us fixed overhead amortized), but takes int16
indices.  The full 100k-row table is not int16-addressable, so the host
splits each core's nodes into two halves and builds a per-half COMPACT
table holding only the ~22k unique rows that half references (remapped
int16 indices).  Each unique row is uploaded once; every per-sample random
fetch still happens on-device from the compact table.

Per-core device pipeline (12500 nodes = 98 tiles of 128, 2 halves x 49):
  - one dma_gather per 2-tile block (1024 indices — the SWDGE ring holds
    1024 descriptors, larger calls hard-fault; 4 queues round-robin so
    descriptor generation never stalls on ring drain; slot i ->
    (i%128, i//128) matches the (tile, stream) layout exactly)
  - hcT/hpl stream in bf16 via HWDGE; v = hc0 @ W.T in bf16 on the PE
  - scalar engine converts v (PSUM fp32) to bf16 once per tile
  - dot products split across engines (no fused multiply-reduce exists on
    this runtime): DVE multiplies all 5 streams (broadcast 2x pass for the
    4 gathered ones) and segment-reduces 3; ACT reduces the other 2 via
    Copy+accumulate.  Pool runs ONLY gathers — interleaving any tensor op
    there costs ~1us MODIFY_POOL_CONFIG per switch.
"""

import sys

for _p in ("/opt/trn_rl_repo",):
    if _p not in sys.path:
        sys.path.insert(0, _p)

import ml_dtypes
import numpy as np

import concourse.mybir as mybir
import concourse.tile as tile
from concourse import bacc
from concourse.bass_utils import run_bass_kernel_spmd

P = 128  # partitions

_NP_DT = {
    mybir.dt.bfloat16: ml_dtypes.bfloat16,
    mybir.dt.float8e3: ml_dtypes.float8_e3m4,
    mybir.dt.float8e4: ml_dtypes.float8_e4m3,
    mybir.dt.float32: np.float32,
}


class Cfg:
    """Problem geometry. Full-size defaults; shrink for CoreSim validation."""

    def __init__(self, n_table=100000, nodes_per_core=12500, d=512, s=4,
                 n_cores=8, call_tiles=2, table_pad=32768,
                 mm_dtype=mybir.dt.bfloat16,      # matmul operands (hcT, W)
                 g_dtype=mybir.dt.bfloat16,       # gather table
                 hpl_dtype=mybir.dt.bfloat16):    # h_pl stream
        self.n_table = n_table
        self.nodes_per_core = nodes_per_core
        self.d = d
        self.s = s
        self.n_cores = n_cores
        self.call_tiles = call_tiles    # node-tiles per dma_gather call
        self.table_pad = table_pad      # compact-table row allocation
        self.mm_dtype = mm_dtype
        self.g_dtype = g_dtype
        self.hpl_dtype = hpl_dtype
        self.tiles = -(-nodes_per_core // P)        # ceil
        self.npad = self.tiles * P
        self.kc = d // P                # contraction chunks
        # two halves of tiles; each must dedup below int16 range
        self.half_tiles = -(-self.tiles // 2)
        self.idx_cols_half = self.half_tiles * s * P // 16

    def blocks(self):
        """(t0_local, n_tiles) call blocks within one half."""
        out = []
        t = 0
        while t < self.half_tiles:
            nt = min(self.call_tiles, self.half_tiles - t)
            out.append((t, nt))
            t += nt
        return out


FULL = Cfg()


def build_nc(cfg: Cfg):
    D, S, KC, ST = cfg.d, cfg.s, cfg.kc, cfg.call_tiles
    mmdt, gdt, pldt = cfg.mm_dtype, cfg.g_dtype, cfg.hpl_dtype
    f32 = mybir.dt.float32

    nc = bacc.Bacc("TRN2", target_bir_lowering=False, debug=False,
                   num_swdge_queues=4)
    hcA = nc.dram_tensor("hcA", [cfg.table_pad, D], gdt,
                         kind="ExternalInput").ap()
    hcB = nc.dram_tensor("hcB", [cfg.table_pad, D], gdt,
                         kind="ExternalInput").ap()
    hcT = nc.dram_tensor("hcT", [D, cfg.npad], mmdt, kind="ExternalInput").ap()
    hpl = nc.dram_tensor("hpl", [cfg.npad, D], pldt, kind="ExternalInput").ap()
    idx = nc.dram_tensor("idx", [P, 2 * cfg.idx_cols_half], mybir.dt.int16,
                         kind="ExternalInput").ap()
    wt = nc.dram_tensor("wt", [D, D], mmdt, kind="ExternalInput").ap()
    bb = nc.dram_tensor("bb", [P, 1], f32, kind="ExternalInput").ap()
    out = nc.dram_tensor("out", [P, cfg.tiles * (S + 1)], f32,
                         kind="ExternalOutput").ap()

    with tile.TileContext(nc) as tc:
        with (
            tc.tile_pool(name="const", bufs=1) as cpool,
            tc.tile_pool(name="hcT", bufs=2) as hcT_pool,
            tc.tile_pool(name="hpl", bufs=2) as hpl_pool,
            tc.tile_pool(name="g", bufs=4) as g_pool,
            tc.tile_pool(name="vbf", bufs=8) as v_pool,
            tc.tile_pool(name="prod", bufs=6) as prod_pool,
            tc.tile_pool(name="psum", bufs=8, space="PSUM") as psum_pool,
        ):
            idx_sb = cpool.tile([P, 2 * cfg.idx_cols_half], mybir.dt.int16)
            nc.sync.dma_start(out=idx_sb[:], in_=idx[:])
            # W.T resident: free layout (c, d) — chunk c covers contraction
            # rows c*128..c*128+127.
            wt_sb = cpool.tile([P, KC * D], mmdt)
            nc.sync.dma_start(
                out=wt_sb[:].rearrange("p (c d) -> p c d", c=KC),
                in_=wt.rearrange("(c p) d -> p c d", p=P))
            # per-engine score tiles (ACT writes sc_a, DVE writes sc_d) so
            # the two accumulation streams never alias one tile
            sc_a = cpool.tile([P, cfg.tiles * 2], f32)
            sc_d = cpool.tile([P, cfg.tiles * 3], f32)
            dump = cpool.tile([P, D], mybir.dt.bfloat16)  # discarded ACT out

            call_no = 0
            for half, table in ((0, hcA), (1, hcB)):
                for t0_loc, nt in cfg.blocks():
                    t0 = half * cfg.half_tiles + t0_loc
                    nt = min(nt, cfg.tiles - t0)
                    if nt <= 0:
                        break
                    n_idx = nt * S * P
                    col0 = half * cfg.idx_cols_half + t0_loc * S * P // 16
                    # hcT block [D, nt*128] -> SBUF free layout (c, n_local)
                    hcT_sb = hcT_pool.tile([P, KC * ST * P], mmdt, tag="hcT")
                    nc.sync.dma_start(
                        out=hcT_sb[:, : KC * nt * P].rearrange(
                            "p (c n) -> p c n", c=KC),
                        in_=hcT[:, t0 * P:(t0 + nt) * P].rearrange(
                            "(c p) n -> p c n", p=P),
                    )
                    g_sb = g_pool.tile([P, ST * S * D], gdt, tag="g")
                    nc.gpsimd.dma_gather(
                        out_ap=g_sb[:, : nt * S * D].rearrange(
                            "p (k d) -> p k d", d=D),
                        in_ap=table[:],
                        idxs_ap=idx_sb[:, col0:col0 + n_idx // 16],
                        num_idxs=n_idx,
                        num_idxs_reg=n_idx,
                        elem_size=D,
                        queue_num=call_no % 4,
                    )
                    call_no += 1
                    # h_pl rows for the whole block in one DMA
                    hpl_sb = hpl_pool.tile([P, ST * D], pldt, tag="hpl")
                    nc.sync.dma_start(
                        out=hpl_sb[:, : nt * D].rearrange(
                            "p (t d) -> p t d", t=nt),
                        in_=hpl[t0 * P:(t0 + nt) * P, :].rearrange(
                            "(t p) d -> p t d", p=P),
                    )
                    for j in range(nt):
                        t = t0 + j
                        # v = hc0_tile @ W.T via KC accumulating matmuls
                        v_ps = psum_pool.tile([P, D], f32, space="PSUM",
                                              tag="v_ps")
                        for c in range(KC):
                            off = (c * nt + j) * P
                            nc.tensor.matmul(
                                out=v_ps[:],
                                lhsT=hcT_sb[:, off:off + P],
                                rhs=wt_sb[:, c * D:(c + 1) * D],
                                start=(c == 0),
                                stop=(c == KC - 1),
                            )
                        # bf16 copy of v so the DVE runs in 16-bit mode
                        v_bf = v_pool.tile([P, D], mybir.dt.bfloat16,
                                           tag="v_bf")
                        nc.scalar.activation(
                            v_bf[:], v_ps[:],
                            mybir.ActivationFunctionType.Copy)
                        # DVE: h_pl product (stream 0)
                        prod0 = prod_pool.tile([P, D], mybir.dt.bfloat16,
                                               tag="prod0")
                        nc.vector.tensor_mul(prod0[:],
                                             hpl_sb[:, j * D:(j + 1) * D],
                                             v_bf[:])
                        # DVE: all 4 gathered products in one 2x pass
                        prod4 = prod_pool.tile([P, S * D], mybir.dt.bfloat16,
                                               tag="prod4")
                        nc.vector.tensor_tensor(
                            out=prod4[:].rearrange("p (s d) -> p s d", s=S),
                            in0=g_sb[:, j * S * D:(j + 1) * S * D].rearrange(
                                "p (s d) -> p s d", s=S),
                            in1=v_bf[:].unsqueeze(1).broadcast_to((P, S, D)),
                            op=mybir.AluOpType.mult)
                        # ACT: reduce streams 0 (h_pl) and 1
                        nc.scalar.activation(
                            dump[:], prod0[:],
                            mybir.ActivationFunctionType.Copy,
                            accum_out=sc_a[:, 2 * t:2 * t + 1])
                        nc.scalar.activation(
                            dump[:], prod4[:, :D],
                            mybir.ActivationFunctionType.Copy,
                            accum_out=sc_a[:, 2 * t + 1:2 * t + 2])
                        # DVE: fold streams 2..4 in half at 2x, then the 1x
                        # segmented reduce only sees half the elements
                        fold = prod_pool.tile([P, 3 * (D // 2)],
                                              mybir.dt.bfloat16, tag="fold")
                        h = D // 2
                        nc.vector.tensor_tensor(
                            out=fold[:].rearrange("p (s d) -> p s d", s=3),
                            in0=prod4[:, D:].rearrange(
                                "p (s d) -> p s d", d=D)[:, :, :h],
                            in1=prod4[:, D:].rearrange(
                                "p (s d) -> p s d", d=D)[:, :, h:],
                            op=mybir.AluOpType.add)
                        nc.vector.tensor_reduce(
                            out=sc_d[:, 3 * t:3 * t + 3],
                            in_=fold[:].rearrange("p (s d) -> p s d", s=3),
                            axis=mybir.AxisListType.X,
                            op=mybir.AluOpType.add)
            # bias is added host-side during assembly (exact for any b);
            # keeping it off-device trims the kernel tail
            nc.sync.dma_start(out=out[:, : cfg.tiles * 2], in_=sc_a[:])
            nc.sync.dma_start(out=out[:, cfg.tiles * 2:], in_=sc_d[:])
    nc.compile()
    return nc


def make_in_maps(cfg: Cfg, h_c, h_pl, sample_list, W, b):
    """Host-side sharding: full inputs -> per-core input dicts."""
    D, S = cfg.d, cfg.s
    gnp = _NP_DT[cfg.g_dtype]
    mnp = _NP_DT[cfg.mm_dtype]
    pnp = _NP_DT[cfg.hpl_dtype]
    hc0 = np.asarray(h_c, np.float32)[0]
    hpl0 = np.asarray(h_pl, np.float32)[0]
    smp = np.asarray(sample_list)
    W0 = np.asarray(W, np.float32)[0]
    bval = float(np.asarray(b, np.float32).reshape(-1)[0])

    hc_g = hc0.astype(gnp)                             # quantized gather rows
    hcT = np.ascontiguousarray(hc0.T.astype(mnp))      # [D, N]
    wt = np.ascontiguousarray(W0.T.astype(mnp))        # wt[d, e] = W[e, d]
    b_bcast = np.full((P, 1), bval, np.float32)
    half_nodes = cfg.half_tiles * P

    in_maps = []
    for c in range(cfg.n_cores):
        lo = c * cfg.nodes_per_core
        hi = lo + cfg.nodes_per_core
        hcT_s = np.zeros((D, cfg.npad), mnp)
        hcT_s[:, : cfg.nodes_per_core] = hcT[:, lo:hi]
        hpl_s = np.zeros((cfg.npad, D), pnp)
        hpl_s[: cfg.nodes_per_core] = hpl0[lo:hi].astype(pnp)
        idx_s = np.zeros((S, cfg.npad), np.int64)
        idx_s[:, : cfg.nodes_per_core] = smp[:, lo:hi]

        # per-half compact tables + int16 index lists in call-slot order:
        # flat position ((j*S + s)*128 + p) = sample (s, node j*128+p)
        tables = []
        idx16 = np.zeros((P, 2 * cfg.idx_cols_half), np.int16)
        for h in range(2):
            n0 = h * half_nodes
            n1 = min(n0 + half_nodes, cfg.npad)
            # [tiles_in_half, S, P] -> flat in (j, s, p) order
            m = (idx_s[:, n0:n1].reshape(S, -1, P)
                 .transpose(1, 0, 2).reshape(-1))
            uniq, inv = np.unique(m, return_inverse=True)
            assert len(uniq) <= cfg.table_pad, (c, h, len(uniq))
            tbl = np.zeros((cfg.table_pad, D), gnp)
            tbl[: len(uniq)] = hc_g[uniq]
            tables.append(tbl)
            # pad the half's flat list to the full half_tiles extent
            flat = np.zeros(cfg.half_tiles * S * P, np.int16)
            flat[: len(inv)] = inv.astype(np.int16)
            wrapped = flat.reshape(-1, 16).T                 # [16, cols]
            cs = h * cfg.idx_cols_half
            idx16[:, cs:cs + cfg.idx_cols_half] = np.tile(
                wrapped, (8, 1))
        in_maps.append({
            "hcA": tables[0], "hcB": tables[1], "hcT": hcT_s, "hpl": hpl_s,
            "idx": idx16, "wt": wt, "bb": b_bcast,
        })
    return in_maps


def assemble_output(cfg: Cfg, outs, bval=0.0):
    """Per-core 'out' arrays [P, TILES*(S+1)] -> full logits [1, N + 2*S*N].

    Device layout: out[:, :2T] = sc_a (streams 0,1 per tile),
    out[:, 2T:] = sc_d (streams 2,3,4 per tile)."""
    S = cfg.s
    T = cfg.tiles
    n = cfg.nodes_per_core * cfg.n_cores
    sc1 = np.empty((n,), np.float32)
    sc2 = np.empty((S, n), np.float32)
    for c in range(cfg.n_cores):
        oa = outs[c][:, : 2 * T].reshape(P, T, 2)
        od = outs[c][:, 2 * T:].reshape(P, T, 3)
        o = (np.concatenate([oa, od], axis=2).transpose(2, 1, 0)
             .reshape(S + 1, cfg.npad)[:, : cfg.nodes_per_core])
        lo = c * cfg.nodes_per_core
        sc1[lo:lo + cfg.nodes_per_core] = o[0]
        sc2[:, lo:lo + cfg.nodes_per_core] = o[1:]
    flat = sc2.reshape(-1)
    res = np.concatenate([sc1, flat, flat])[None, :] + np.float32(bval)
    return res.astype(np.float32)


_NC_CACHE = {}


def _get_nc(cfg: Cfg):
    key = (cfg.n_table, cfg.nodes_per_core, cfg.d, cfg.s, cfg.call_tiles,
           cfg.mm_dtype, cfg.g_dtype, cfg.hpl_dtype)
    if key not in _NC_CACHE:
        _NC_CACHE[key] = build_nc(cfg)
    return _NC_CACHE[key]


def run_on_hw(cfg: Cfg, inputs, trace=False, trace_kwargs={}):
    nc = _get_nc(cfg)
    in_maps = make_in_maps(cfg, **inputs)
    res = run_bass_kernel_spmd(nc, in_maps, core_ids=list(range(cfg.n_cores)),
                               trace=trace, trace_kwargs=trace_kwargs)
    bval = float(np.asarray(inputs["b"], np.float32).reshape(-1)[0])
    out = assemble_output(cfg, [r["out"] for r in res.results], bval)
    return out, res


def kernel(h_c, h_pl, sample_list, W, b):
    inputs = dict(h_c=h_c, h_pl=h_pl, sample_list=sample_list, W=W, b=b)
    out, _ = run_on_hw(FULL, inputs, trace=False)
    return out
